# revision 1
# baseline (speedup 1.0000x reference)
"""Trainium2 Bass kernel for CustomConv: 3x3 conv (pad=1, stride=1) + bias + ReLU.

Input  prev_a  [32, 56, 56, 128] f32 (NHWC)
       filter_w [3, 3, 128, 256] f32 (HWIO)
       filter_b [1, 1, 1, 256]   f32
Output [32, 56, 56, 256] f32

Strategy: data-parallel over batch (4 images per core on 8 cores).
Host pre-transposes to NCHW with a 1-px zero-padded ring so each of the
9 filter taps is a strided SBUF view; conv = 9 accumulated matmuls per
output tile (contraction over the 128 input channels on the partition
dim). Matmuls run in float32r (fp32 with 11 mantissa bits) which
streams at full PE rate; bias+ReLU is fused on the scalar engine.
"""
import numpy as np

import concourse.bass as bass
import concourse.tile as tile
from concourse import bacc, mybir
from concourse import bass_utils

# Disable walrus birsim (compile-time simulation). Produces an identical
# NEFF; avoids minutes of per-element fp32r software casting at compile.
_orig_run_command = bass_utils.run_command


def _no_birsim_run_command(argv, **kwargs):
    argv = ["--enable-birsim=false" if a == "--enable-birsim=true" else a
            for a in argv]
    return _orig_run_command(argv, **kwargs)


bass_utils.run_command = _no_birsim_run_command

N_CORES = 8
IMG_PER_CORE = 4
H = 56          # output spatial
HP = 58         # padded input spatial
CIN = 128
COUT = 256
TAPS = [(dy, dx) for dy in range(3) for dx in range(3)]
RG = 7          # row groups per image
RG_ROWS = 8     # output rows per group
NFREE = RG_ROWS * H  # 448 positions per matmul (<= 512 PSUM bank)

TRACE = False
TRACE_KWARGS = {}
LAST_RESULTS = None
_NC_CACHE = None


def _round_fp32r(a: np.ndarray) -> np.ndarray:
    """Round-to-nearest-even to 11 explicit mantissa bits (fp32r grid)."""
    b = np.ascontiguousarray(a, dtype=np.float32).view(np.uint32)
    drop = 12
    bias = ((b >> drop) & np.uint32(1)) + np.uint32((1 << (drop - 1)) - 1)
    b = (b + bias) & np.uint32((~((1 << drop) - 1)) & 0xFFFFFFFF)
    return b.view(np.float32)


def _build():
    nc = bacc.Bacc("TRN2", debug=False, target_bir_lowering=False,
                   num_devices=N_CORES)
    x_d = nc.dram_tensor("x", [IMG_PER_CORE, CIN, HP, HP],
                         mybir.dt.float32r, kind="ExternalInput")
    w_d = nc.dram_tensor("w", [CIN, 9, COUT],
                         mybir.dt.float32r, kind="ExternalInput")
    b_d = nc.dram_tensor("b", [CIN, 2], mybir.dt.float32, kind="ExternalInput")
    o_d = nc.dram_tensor("o", [IMG_PER_CORE, 2, 128, H * H],
                         mybir.dt.float32, kind="ExternalOutput")

    with tile.TileContext(nc) as tc:
        with (tc.tile_pool(name="wb", bufs=1) as wbp,
              tc.tile_pool(name="x", bufs=IMG_PER_CORE) as xp,
              tc.tile_pool(name="o", bufs=4) as op,
              tc.tile_pool(name="ps", bufs=4, space="PSUM") as pp):
            wt = wbp.tile([CIN, 9, COUT], mybir.dt.float32r)
            nc.sync.dma_start(wt[:], w_d.ap())
            bt = wbp.tile([CIN, 2], mybir.dt.float32)
            nc.sync.dma_start(bt[:], b_d.ap())
            xts = []
            for img in range(IMG_PER_CORE):
                xt = xp.tile([CIN, HP, HP], mybir.dt.float32r, tag="ximg")
                nc.sync.dma_start(xt[:], x_d.ap()[img])
                xts.append(xt)

            for img in range(IMG_PER_CORE):
                for rg in range(RG):
                    r0 = rg * RG_ROWS
                    for j in range(2):
                        ps = pp.tile([128, NFREE], mybir.dt.float32)
                        for t, (dy, dx) in enumerate(TAPS):
                            nc.tensor.matmul(
                                ps[:],
                                wt[:, t, j * 128:(j + 1) * 128],
                                xts[img][:, r0 + dy: r0 + dy + RG_ROWS,
                                         dx: dx + H],
                                start=(t == 0), stop=(t == 8),
                            )
                        ot = op.tile([128, NFREE], mybir.dt.float32)
                        nc.scalar.activation(
                            ot[:], ps[:], mybir.ActivationFunctionType.Relu,
                            bias=bt[:, j:j + 1])
                        nc.sync.dma_start(
                            o_d.ap()[img, j, :, r0 * H:(r0 + RG_ROWS) * H],
                            ot[:])
    nc.compile()
    return nc


def kernel(prev_a, filter_w, filter_b):
    global LAST_RESULTS, _NC_CACHE
    from concourse.bass_utils import run_bass_kernel_spmd

    prev_a = np.asarray(prev_a, dtype=np.float32)
    filter_w = np.asarray(filter_w, dtype=np.float32)
    filter_b = np.asarray(filter_b, dtype=np.float32)

    n = prev_a.shape[0]
    xpad = np.zeros((n, CIN, HP, HP), dtype=np.float32)
    xpad[:, :, 1:1 + H, 1:1 + H] = prev_a.transpose(0, 3, 1, 2)
    xpad = _round_fp32r(xpad)
    w = _round_fp32r(filter_w.transpose(2, 0, 1, 3).reshape(CIN, 9, COUT))
    b = np.ascontiguousarray(filter_b.reshape(2, 128).T)

    if _NC_CACHE is None:
        _NC_CACHE = _build()
    nc = _NC_CACHE

    in_maps = [
        {"x": np.ascontiguousarray(xpad[c * IMG_PER_CORE:(c + 1) * IMG_PER_CORE]),
         "w": w, "b": b}
        for c in range(N_CORES)
    ]
    LAST_RESULTS = run_bass_kernel_spmd(
        nc, in_maps, core_ids=list(range(N_CORES)), trace=TRACE,
        **TRACE_KWARGS)

    outs = []
    for c in range(N_CORES):
        o = LAST_RESULTS.results[c]["o"]  # [4, 2, 128, 3136]
        outs.append(o.reshape(IMG_PER_CORE, COUT, H, H).transpose(0, 2, 3, 1))
    return np.ascontiguousarray(np.concatenate(outs, axis=0))
